# revision 56
# baseline (speedup 1.0000x reference)
"""Trainium2 Bass kernel for nn_Attention_29935922053658 (sparse frame attention).

Sharding: data-parallel over batch B=8 -> 8 NeuronCores (1 batch each).
Per-core: fused qkv-proj + frame-local attention (196-token frames, cls token
attends globally) + out-proj, streamed per frame with bf16 matmuls / fp32 accum.

v5: cls attention piggybacks on the frame score/AV matmuls (cls-q as a 197th
query column; 15x self-term overcount corrected at the epilogue with
host-precomputed constants), denominator reciprocals via a fused Ln/Exp(-x)
pair, pair-level normalization, and a 1-pair software pipeline so the PE never
drains. x and the weights are pre-cast to bf16 on the host.
"""

import sys
import types
import json
import math

for _p in ("/opt/trn_rl_repo", "/root/.axon_site"):
    if _p not in sys.path:
        sys.path.insert(0, _p)

import numpy as np

# ---------------------------------------------------------------------------
# Environment shims (required under the axon-proxied PJRT runtime):
#  1. antenv.axon_hooks registry (missing in this image) so trace=True can work.
#  2. Split >2 sync-waits off Drain instructions — this walrus build's CoreV3
#     codegen rejects them ("Too many sync wait commands").
#  3. upload_artifacts: no artifact bucket in this container.
# ---------------------------------------------------------------------------


def _install_shims():
    import antenv

    if "antenv.axon_hooks" not in sys.modules:
        m = types.ModuleType("antenv.axon_hooks")
        m._hook = None

        def set_axon_ntff_profile_hook(h):
            m._hook = h

        def get_axon_ntff_profile_hook():
            return m._hook

        m.set_axon_ntff_profile_hook = set_axon_ntff_profile_hook
        m.get_axon_ntff_profile_hook = get_axon_ntff_profile_hook
        sys.modules["antenv.axon_hooks"] = m
        antenv.axon_hooks = m
        try:
            from trn_agent_boot.trn_boot import _ntff_profile_via_ctypes

            hook = _ntff_profile_via_ctypes("/opt/axon/libaxon_pjrt.so")
            if hook is not None:
                m._hook = hook
        except Exception:
            pass

    import concourse.bass_utils as bu
    import concourse.bass2jax as b2j

    if not getattr(bu, "_drain_patch_installed", False):
        bu._drain_patch_installed = True
        bu.upload_artifacts = lambda tmpdir: "local://" + str(tmpdir)

        _orig = b2j.compile_bir_kernel

        def _patched_compile(ant_bir_str, compile_dir, neff_name="file.neff"):
            # This walrus build's codegen accepts at most ONE sync-wait per
            # instruction; hoist extras onto chained same-engine NoOps.
            d = json.loads(ant_bir_str)
            changed = False
            for fn in d.get("functions", []):
                for blk in fn.get("blocks", []):
                    insts = blk.get("instructions", [])
                    out = []
                    for ins in insts:
                        si = ins.get("sync_info") or {}
                        waits = si.get("on_wait") or []
                        if len(waits) > 1:
                            for ci, w in enumerate(waits[:-1]):
                                out.append(
                                    {
                                        "debug": ins.get("debug", 0),
                                        "engine": ins["engine"],
                                        "ins": [],
                                        "outs": [],
                                        "name": ins["name"] + f"-ws{ci}",
                                        "opcode": "NoOp",
                                        "sync_info": {
                                            "on_update": [],
                                            "on_wait": [w],
                                        },
                                    }
                                )
                            si["on_wait"] = waits[-1:]
                            changed = True
                        out.append(ins)
                    blk["instructions"] = out
            if changed:
                ant_bir_str = json.dumps(d).encode()
            return _orig(ant_bir_str, compile_dir, neff_name=neff_name)

        b2j.compile_bir_kernel = _patched_compile


_install_shims()

import concourse.bass as bass
import concourse.mybir as mybir
import concourse.tile as tile
from concourse.bass_utils import run_bass_kernel_spmd

f32 = mybir.dt.float32
bf16 = mybir.dt.bfloat16
AF = mybir.ActivationFunctionType
AX = mybir.AxisListType
ALU = mybir.AluOpType

# Problem constants (hardcoded per spec)
N_SEQ = 3137
N_SEQ_PAD = 3152
DIM = 512
H = 8
DH = 64
F = 16
NF = 196  # tokens per frame
NK = 197  # keys per frame block (frame + cls); also queries per block (+cls q)
PAIR = 2 * NF  # 392
PK = 2 * NK  # 394
N_CORES = 8

TOK_CHUNKS = [(0, 128), (128, 68)]


def build_kernel():
    nc = bass.Bass()
    x_d = nc.dram_tensor("x", [N_SEQ_PAD, DIM], bf16, kind="ExternalInput")
    wqkv_d = nc.dram_tensor("wqkv", [DIM, 3 * DIM], bf16, kind="ExternalInput")
    wout_d = nc.dram_tensor("wout", [DIM, DIM], bf16, kind="ExternalInput")
    bout_d = nc.dram_tensor("bout", [1, DIM], f32, kind="ExternalInput")
    ident_d = nc.dram_tensor("ident", [128, 128], bf16, kind="ExternalInput")
    ident32_d = nc.dram_tensor("ident32", [80, 65], f32, kind="ExternalInput")
    ind8_d = nc.dram_tensor("ind8", [8, DIM], bf16, kind="ExternalInput")
    qkTcls_d = nc.dram_tensor("qkTcls", [128, 8], bf16, kind="ExternalInput")
    vextcls_d = nc.dram_tensor("vextcls", [1, 8 * 65], bf16, kind="ExternalInput")
    corr8_d = nc.dram_tensor("corr8", [8, 65], f32, kind="ExternalInput")
    out_d = nc.dram_tensor("out", [N_SEQ, DIM], f32, kind="ExternalOutput")

    with tile.TileContext(nc) as tc:
        with (
            tc.tile_pool(name="const", bufs=1) as cpool,
            tc.tile_pool(name="work", bufs=2) as wpool,
            tc.tile_pool(name="at", bufs=4) as apool,
            tc.tile_pool(name="big_ps", bufs=2, space="PSUM") as big_ps,
            tc.tile_pool(name="attn_ps", bufs=3, space="PSUM") as attn_ps,
            tc.tile_pool(name="po_ps", bufs=2, space="PSUM") as po_ps,
            tc.tile_pool(name="rsb_ps", bufs=1, space="PSUM") as rsb_ps,
        ):
            # ---------------- preamble: DMAs in latency order ----------------
            ident = cpool.tile([128, 128], bf16, name="ident", tag="ident")
            nc.sync.dma_start(out=ident[:], in_=ident_d[:])
            # tiny cls constants first: frame 0's scores/po need them early
            qkTcls = cpool.tile([128, 8], bf16, name="qkTcls", tag="qkTcls")
            nc.sync.dma_start(out=qkTcls[:], in_=qkTcls_d[:])
            vextcls = cpool.tile([1, 8 * 65], bf16, name="vextcls", tag="vextcls")
            nc.sync.dma_start(out=vextcls[:], in_=vextcls_d[:])

            def stage_load(fp):
                """DMA the 2*NF x rows of pair fp (already bf16 in DRAM)."""
                xbf = []
                pr0 = 1 + fp * PAIR
                for fl in range(2):
                    for t, (t0, tn) in enumerate(TOK_CHUNKS):
                        i = 2 * fl + t
                        pt = 128 if t == 0 else 80  # pad rows to /16 for xbar
                        xb = wpool.tile(
                            [pt, DIM], bf16, name=f"xbf_{i}", tag=f"xbf_{i}"
                        )
                        if t == 1:
                            nc.gpsimd.memset(xb[64:80, :], 0.0)
                        nc.sync.dma_start(
                            out=xb[0:tn, :],
                            in_=x_d[pr0 + fl * NF + t0 : pr0 + fl * NF + t0 + tn, :],
                        )
                        xbf.append(xb)
                return xbf

            xbf_cur = stage_load(0)

            # v_ext ring: 2 frames in flight; ones cols + cls row written once
            # (early: frame 0's po matmuls need the cls row).
            vext_ring = []
            for r in range(2):
                pair_t = []
                for t, (t0, tn) in enumerate(TOK_CHUNKS):
                    pn = 128 if t == 0 else 69
                    vx = cpool.tile(
                        [pn, 8 * 65], bf16, name=f"vext_r{r}_{t}", tag=f"vext_r{r}_{t}"
                    )
                    nc.gpsimd.memset(
                        vx[:].rearrange("p (h c) -> p h c", c=65)[:, :, 64:65], 1.0
                    )
                    if t == 1:
                        nc.sync.dma_start(out=vx[68:69, :], in_=vextcls[:])
                    pair_t.append(vx)
                vext_ring.append(pair_t)

            # q/k weight columns first: the qk projection needs them ~5us
            # before the first v projection does.
            wqkv_bf = []
            for c in range(4):
                tb = cpool.tile([128, 3 * DIM], bf16, name=f"wqkv{c}", tag=f"wqkv{c}")
                nc.sync.dma_start(
                    out=tb[:, 0 : 2 * DIM], in_=wqkv_d[c * 128 : (c + 1) * 128, 0 : 2 * DIM]
                )
                wqkv_bf.append(tb)
            for c in range(4):
                nc.sync.dma_start(
                    out=wqkv_bf[c][:, 2 * DIM : 3 * DIM],
                    in_=wqkv_d[c * 128 : (c + 1) * 128, 2 * DIM : 3 * DIM],
                )

            def stage_transpose(fp, xbf):
                xTs = []
                for c in range(4):
                    ps_t = attn_ps.tile([128, PAIR], bf16, name="ps_t", tag="attn")
                    for fl in range(2):
                        for t, (t0, tn) in enumerate(TOK_CHUNKS):
                            g0 = fl * NF + t0
                            nc.tensor.transpose(
                                ps_t[:, g0 : g0 + tn],
                                xbf[2 * fl + t][0:tn, c * 128 : (c + 1) * 128],
                                ident[0:tn, 0:tn],
                            )
                    xt = wpool.tile([128, PAIR], bf16, name=f"xT_{c}", tag=f"xT_{c}")
                    nc.vector.tensor_copy(xt[:], ps_t[:])
                    xTs.append(xt)
                return xTs

            xT_cur = stage_transpose(0, xbf_cur)

            wout_bf = []
            for c in range(4):
                tb = cpool.tile([128, DIM], bf16, name=f"wout{c}", tag=f"wout{c}")
                nc.sync.dma_start(out=tb[:], in_=wout_d[c * 128 : (c + 1) * 128, :])
                wout_bf.append(tb)

            # small constants
            corr8 = cpool.tile([8, 65], f32, name="corr8", tag="corr8")
            nc.sync.dma_start(out=corr8[:], in_=corr8_d[:])
            ident32 = cpool.tile([80, 65], f32, name="ident32", tag="ident32")
            nc.sync.dma_start(out=ident32[:], in_=ident32_d[:])
            ind8 = cpool.tile([8, DIM], bf16, name="ind8", tag="ind8")
            nc.sync.dma_start(out=ind8[:], in_=ind8_d[:])
            bout_sb = cpool.tile([1, DIM], f32, name="bout", tag="bout")
            nc.sync.dma_start(out=bout_sb[:], in_=bout_d[:])

            ones_row = cpool.tile([1, 128], f32, name="ones_row", tag="ones_row")
            nc.gpsimd.memset(ones_row[:], 1.0)
            neg1 = cpool.tile([8, 1], f32, name="neg1", tag="neg1")
            nc.gpsimd.memset(neg1[:], -1.0)

            ps_b = big_ps.tile([128, DIM], f32, name="big", tag="big")
            nc.tensor.matmul(
                ps_b[:], lhsT=ones_row[:], rhs=bout_sb[:], start=True, stop=True
            )
            bout_bc = cpool.tile([128, DIM], f32, name="bout_bc", tag="bout_bc")
            nc.vector.tensor_copy(bout_bc[:], ps_b[:])

            # ---------------- static ring tiles ----------------
            # kq ring: [128, PK] bf16 per m-chunk; cols 196/393 hold the cls
            # q (m<4) / k (m>=4) vector, written once here.
            kq_ring = []
            for r in range(3):
                ring = []
                for m in range(8):
                    t = cpool.tile(
                        [128, PK], bf16, name=f"kq_r{r}_m{m}", tag=f"kq_r{r}_m{m}"
                    )
                    nc.scalar.copy(t[:, NF : NF + 1], qkTcls[:, m : m + 1])
                    nc.scalar.copy(t[:, NK + NF : NK + NF + 1], qkTcls[:, m : m + 1])
                    ring.append(t)
                kq_ring.append(ring)

            # cls accumulation columns: col h*F+fi = head-h contribution of
            # frame fi ([num(64) | den] on partitions 0..64).
            cls_cols = cpool.tile([65, H * F], f32, name="cls_cols", tag="cls_cols")

            # ---------------- stage functions ----------------
            def stage_qk(fp, xT):
                kq = kq_ring[fp % 3]
                for m in range(8):
                    ps_p = attn_ps.tile([128, PAIR], f32, name="ps_p", tag="attn")
                    for c in range(4):
                        nc.tensor.matmul(
                            ps_p[:],
                            lhsT=wqkv_bf[c][:, m * 128 : (m + 1) * 128],
                            rhs=xT[c][:, 0:PAIR],
                            start=(c == 0),
                            stop=(c == 3),
                        )
                    nc.vector.tensor_copy(
                        kq[m][:, 0:PK].rearrange("p (f k) -> p f k", k=NK)[:, :, 0:NF],
                        ps_p[:, 0:PAIR].rearrange("p (f k) -> p f k", k=NF),
                    )
                return kq

            def stage_v(fi, xT):
                fl = fi % 2
                xbase = fl * NF
                vext = vext_ring[fi % 2]
                for t, (t0, tn) in enumerate(TOK_CHUNKS):
                    ps_v = big_ps.tile([tn, DIM], f32, name="ps_v", tag="big")
                    for c in range(4):
                        nc.tensor.matmul(
                            ps_v[:],
                            lhsT=xT[c][:, xbase + t0 : xbase + t0 + tn],
                            rhs=wqkv_bf[c][:, 2 * DIM : 3 * DIM],
                            start=(c == 0),
                            stop=(c == 3),
                        )
                    nc.vector.tensor_copy(
                        vext[t][0:tn, :].rearrange("p (h c) -> p h c", c=65)[
                            :, :, 0:64
                        ],
                        ps_v[:].rearrange("p (h c) -> p h c", c=64),
                    )
                return vext

            def attn_head(fi, h, kq, vext, s8, attnT):
                fl = fi % 2
                kbase = fl * NK
                hc = h // 2
                r = (h % 2) * 64
                kT = kq[4 + hc]
                qT = kq[hc]
                ps_s = attn_ps.tile([128, PK], f32, name="ps_s", tag="attn")
                nc.tensor.matmul(
                    ps_s[:, 0:NK],
                    lhsT=kT[r : r + 64, kbase : kbase + 128],
                    rhs=qT[r : r + 64, kbase : kbase + NK],
                    start=True,
                    stop=True,
                )
                nc.tensor.matmul(
                    ps_s[0:69, NK:PK],
                    lhsT=kT[r : r + 64, kbase + 128 : kbase + NK],
                    rhs=qT[r : r + 64, kbase : kbase + NK],
                    start=True,
                    stop=True,
                )
                aT = apool.tile([128, PK], bf16, name="aT", tag="aT")
                nc.scalar.activation(aT[:], ps_s[:], AF.Exp)
                po = po_ps.tile([65, NK], f32, name="po", tag="po")
                nc.tensor.matmul(
                    po[:],
                    lhsT=vext[0][:, h * 65 : (h + 1) * 65],
                    rhs=aT[:, 0:NK],
                    start=True,
                    stop=False,
                )
                nc.tensor.matmul(
                    po[:],
                    lhsT=vext[1][0:69, h * 65 : (h + 1) * 65],
                    rhs=aT[0:69, NK:PK],
                    start=False,
                    stop=True,
                )
                nc.vector.tensor_copy(attnT[hc][r : r + 64, :], po[0:64, 0:NF])
                nc.scalar.activation(
                    s8[0:1, h * PAIR + fl * NF : h * PAIR + fl * NF + NF],
                    po[64:65, 0:NF],
                    AF.Ln,
                )
                nc.vector.tensor_copy(
                    cls_cols[0:65, h * F + fi : h * F + fi + 1], po[0:65, NF : NF + 1]
                )

            def recip_exp(s8):
                # s8 row holds ln(den) per (head, frame-of-pair); reshape to
                # [8, PAIR] via DMA, then rs8 = exp(-.) = 1/den in bf16
                s8p = wpool.tile([8, PAIR], f32, name="s8p", tag="s8p")
                nc.sync.dma_start(out=s8p[:], in_=s8[0:1, :])
                rs8 = wpool.tile([8, PAIR], bf16, name="rs8", tag="rs8")
                nc.scalar.activation(rs8[:], s8p[:], AF.Exp, scale=neg1[:])
                return rs8

            def stage_norm(rs8, attnT_a, attnT_b, c):
                ps_r = rsb_ps.tile([128, PAIR], f32, name="ps_r", tag="rsb")
                nc.tensor.matmul(
                    ps_r[:],
                    lhsT=ind8[:, c * 128 : (c + 1) * 128],
                    rhs=rs8[:],
                    start=True,
                    stop=True,
                )
                nc.vector.tensor_mul(attnT_a[c][:], attnT_a[c][:], ps_r[:, 0:NF])
                nc.vector.tensor_mul(attnT_b[c][:], attnT_b[c][:], ps_r[:, NF:PAIR])

            def stage_norm_half(rs8, attnT, c, half):
                # normalize one frame of a pair (used to drain the last pair)
                ps_r = rsb_ps.tile([128, PAIR], f32, name="ps_r", tag="rsb")
                nc.tensor.matmul(
                    ps_r[:, 0:NF],
                    lhsT=ind8[:, c * 128 : (c + 1) * 128],
                    rhs=rs8[:, half * NF : half * NF + NF],
                    start=True,
                    stop=True,
                )
                nc.vector.tensor_mul(attnT[c][:], attnT[c][:], ps_r[:, 0:NF])

            def stage_out(fi, t, attnT):
                r0 = 1 + fi * NF
                t0, tn = TOK_CHUNKS[t]
                ps_o = big_ps.tile([tn, DIM], f32, name="ps_o", tag="big")
                for c in range(4):
                    nc.tensor.matmul(
                        ps_o[:],
                        lhsT=attnT[c][:, t0 : t0 + tn],
                        rhs=wout_bf[c][:],
                        start=(c == 0),
                        stop=(c == 3),
                    )
                o_sb = wpool.tile([tn, DIM], f32, name=f"osb_{t}", tag=f"osb_{t}")
                nc.vector.tensor_add(o_sb[:], ps_o[:], bout_bc[0:tn, :])
                nc.sync.dma_start(out=out_d[r0 + t0 : r0 + t0 + tn, :], in_=o_sb[:])

            # pair-0 qk projection (needs wqkv + pair-0 transposes)
            kq_cur = stage_qk(0, xT_cur)

            # ---------------- main loop: 16 frames, 1-pair pipeline ---------
            pending = None  # (pair, s8row, attnT of frame a, attnT of frame b)
            xbf_next = None
            xT_next = None
            kq_next = None
            s8_pair = None
            attnT_a = None
            prs8 = None
            for fi in range(F):
                fp, fl = fi // 2, fi % 2
                if fl == 0 and fp + 1 < F // 2:
                    xbf_next = stage_load(fp + 1)
                vext = stage_v(fi, xT_cur)
                if fl == 0:
                    s8_pair = wpool.tile([1, H * PAIR], f32, name="s8", tag="s8")
                attnT = [
                    wpool.tile(
                        [128, NF], bf16, name=f"attnT_{c}", tag=f"attnT_{c}", bufs=4
                    )
                    for c in range(4)
                ]
                for h in range(H):
                    attn_head(fi, h, kq_cur, vext, s8_pair, attnT)
                    if pending is not None:
                        pa, pb, ps8 = pending
                        if fl == 0:
                            if h == 0:
                                prs8 = recip_exp(ps8)
                            elif 1 <= h <= 4:
                                stage_norm(prs8, pa, pb, h - 1)
                            elif h == 5:
                                stage_out(2 * fp - 2, 0, pa)
                            elif h == 6:
                                stage_out(2 * fp - 2, 1, pa)
                            elif h == 7:
                                stage_out(2 * fp - 1, 0, pb)
                        else:
                            if h == 0:
                                stage_out(2 * fp - 1, 1, pb)
                    if fl == 1 and fp + 1 < F // 2:
                        if h == 5:
                            xT_next = stage_transpose(fp + 1, xbf_next)
                        elif h == 6:
                            kq_next = stage_qk(fp + 1, xT_next)
                    if fl == 1 and fp == F // 2 - 1:
                        # last pair: drain frame 14's norm/out inside frame
                        # 15's head loop (its half of the dens is complete)
                        if h == 1:
                            prs8_a = recip_exp(s8_pair)
                        elif h in (2, 3):
                            stage_norm_half(prs8_a, attnT_a, 2 * (h - 2), 0)
                            stage_norm_half(prs8_a, attnT_a, 2 * (h - 2) + 1, 0)
                        elif h == 4:
                            stage_out(2 * fp, 0, attnT_a)
                        elif h == 5:
                            stage_out(2 * fp, 1, attnT_a)
                if fl == 0:
                    attnT_a = attnT
                else:
                    pending = (attnT_a, attnT, s8_pair)
                    if fp + 1 < F // 2:
                        xT_cur, kq_cur = xT_next, kq_next

            # drain: frame 15's norm + out-proj, interleaved with the cls
            # epilogue's serial Vector/PE chain so the PE stays fed.
            pa, pb, ps8 = pending
            prs8 = recip_exp(ps8)
            red = wpool.tile([65, 8], f32, name="red", tag="red")
            nc.vector.tensor_reduce(
                red[:],
                cls_cols[:].rearrange("p (h f) -> p h f", f=F),
                axis=AX.X,
                op=ALU.add,
            )
            stage_norm_half(prs8, pb, 0, 1)
            stage_norm_half(prs8, pb, 1, 1)
            ps_tr = po_ps.tile([8, 65], f32, name="ps_tr", tag="po")
            nc.tensor.transpose(ps_tr[:], red[:], ident32[0:65, :])
            stage_norm_half(prs8, pb, 2, 1)
            stage_norm_half(prs8, pb, 3, 1)
            acc = wpool.tile([8, 65], f32, name="acc", tag="acc")
            nc.vector.tensor_sub(acc[:], ps_tr[:], corr8[:])
            rden = wpool.tile([8, 1], f32, name="rden", tag="rden")
            nc.vector.reciprocal(rden[:], acc[:, 64:65])
            cls_n = wpool.tile([8, 64], bf16, name="cls_n", tag="cls_n")
            nc.vector.tensor_scalar_mul(cls_n[:], acc[:, 0:64], rden[:, 0:1])
            stage_out(15, 0, pb)
            ps_t2 = attn_ps.tile([64, 8], bf16, name="ps_t2", tag="attn")
            nc.tensor.transpose(ps_t2[:], cls_n[:], ident[0:8, 0:8])
            attnT_cls = [
                wpool.tile([128, 1], bf16, name=f"aTc{c}", tag=f"aTc{c}")
                for c in range(4)
            ]
            for h in range(8):
                nc.vector.tensor_copy(
                    attnT_cls[h // 2][(h % 2) * 64 : (h % 2) * 64 + 64, :],
                    ps_t2[:, h : h + 1],
                )
            stage_out(15, 1, pb)
            ps_oc = big_ps.tile([1, DIM], f32, name="ps_oc", tag="big")
            for c in range(4):
                nc.tensor.matmul(
                    ps_oc[:],
                    lhsT=attnT_cls[c][:],
                    rhs=wout_bf[c][:],
                    start=(c == 0),
                    stop=(c == 3),
                )
            o_cls = wpool.tile([1, DIM], f32, name="o_cls", tag="o_cls")
            nc.vector.tensor_add(o_cls[:], ps_oc[:], bout_bc[0:1, :])
            nc.sync.dma_start(out=out_d[0:1, :], in_=o_cls[:])

    return nc


_NC_CACHE = {}


def _get_nc():
    if "nc" not in _NC_CACHE:
        _NC_CACHE["nc"] = build_kernel()
    return _NC_CACHE["nc"]


def kernel(x, Wqkv, Wout, bout, f, _trace=False, _trace_kwargs=None):
    assert int(f) == F, f"kernel hardcoded for f={F}, got {f}"
    import ml_dtypes

    bf = ml_dtypes.bfloat16
    x = np.asarray(x, np.float32)
    x_pad = np.zeros((N_CORES, N_SEQ_PAD, DIM), dtype=bf)
    x_pad[:, :N_SEQ] = x.astype(bf)
    Wqkv_s = np.asarray(Wqkv, np.float32).copy()
    Wqkv_s[:, :DIM] *= DH ** -0.5  # fold q scaling into the projection
    Wqkv_bf = Wqkv_s.astype(bf)
    Wout_bf = np.asarray(Wout, np.float32).astype(bf)
    bout2 = np.asarray(bout, np.float32).reshape(1, DIM)

    ident_np = np.eye(128, dtype=bf)
    ident32_np = np.zeros((80, 65), dtype=np.float32)
    ident32_np[:65] = np.eye(65, dtype=np.float32)
    ind8_np = np.zeros((8, DIM), dtype=bf)
    for k in range(8):
        ind8_np[k, k * 64 : (k + 1) * 64] = 1.0

    # host-precomputed cls constants (mirrors the device bf16 dataflow)
    nc = _get_nc()
    in_maps = []
    for i in range(N_CORES):
        xcls_bf = x_pad[i, 0, :].astype(np.float32)  # bf16-quantized cls row
        qkv_cls = xcls_bf @ Wqkv_bf.astype(np.float32)  # [1536] fp32
        qkT_np = np.zeros((128, 8), dtype=bf)
        for m in range(8):
            qkT_np[:, m] = qkv_cls[m * 128 : (m + 1) * 128].astype(bf)
        v_cls_bf = qkv_cls[2 * DIM : 3 * DIM].astype(bf).astype(np.float32)
        vext_np = np.zeros((1, 8 * 65), dtype=bf)
        for hh in range(8):
            vext_np[0, hh * 65 : hh * 65 + 64] = v_cls_bf[hh * 64 : (hh + 1) * 64]
            vext_np[0, hh * 65 + 64] = 1.0
        # self-score from the bf16-quantized q/k (matches frame matmuls)
        qb = qkT_np[:, 0:4].astype(np.float32).T.reshape(DIM)
        kb = qkT_np[:, 4:8].astype(np.float32).T.reshape(DIM)
        s_self = (qb.reshape(8, 64) * kb.reshape(8, 64)).sum(axis=1)  # [8]
        a15 = 15.0 * np.exp(s_self)
        corr_np = np.zeros((8, 65), dtype=np.float32)
        corr_np[:, 0:64] = a15[:, None] * v_cls_bf.reshape(8, 64)
        corr_np[:, 64] = a15
        in_maps.append(
            {
                "x": x_pad[i],
                "wqkv": Wqkv_bf,
                "wout": Wout_bf,
                "bout": bout2,
                "ident": ident_np,
                "ident32": ident32_np,
                "ind8": ind8_np,
                "qkTcls": qkT_np,
                "vextcls": vext_np,
                "corr8": corr_np,
            }
        )
    res = run_bass_kernel_spmd(
        nc,
        in_maps,
        list(range(N_CORES)),
        trace=_trace,
        **(_trace_kwargs or {}),
    )
    out = np.stack([res.results[i]["out"] for i in range(N_CORES)], axis=0)
    if _trace:
        kernel.last_results = res
    return out


# revision 61
# speedup vs baseline: 1.0045x; 1.0045x over previous
"""Trainium2 Bass kernel for nn_Attention_29935922053658 (sparse frame attention).

Sharding: data-parallel over batch B=8 -> 8 NeuronCores (1 batch each).
Per-core: fused qkv-proj + frame-local attention (196-token frames, cls token
attends globally) + out-proj, streamed per frame with bf16 matmuls / fp32 accum.

v5: cls attention piggybacks on the frame score/AV matmuls (cls-q as a 197th
query column; 15x self-term overcount corrected at the epilogue with
host-precomputed constants), denominator reciprocals via a fused Ln/Exp(-x)
pair, pair-level normalization, and a 1-pair software pipeline so the PE never
drains. x and the weights are pre-cast to bf16 on the host.
"""

import sys
import types
import json
import math

for _p in ("/opt/trn_rl_repo", "/root/.axon_site"):
    if _p not in sys.path:
        sys.path.insert(0, _p)

import numpy as np

# ---------------------------------------------------------------------------
# Environment shims (required under the axon-proxied PJRT runtime):
#  1. antenv.axon_hooks registry (missing in this image) so trace=True can work.
#  2. Split >2 sync-waits off Drain instructions — this walrus build's CoreV3
#     codegen rejects them ("Too many sync wait commands").
#  3. upload_artifacts: no artifact bucket in this container.
# ---------------------------------------------------------------------------


def _install_shims():
    import antenv

    if "antenv.axon_hooks" not in sys.modules:
        m = types.ModuleType("antenv.axon_hooks")
        m._hook = None

        def set_axon_ntff_profile_hook(h):
            m._hook = h

        def get_axon_ntff_profile_hook():
            return m._hook

        m.set_axon_ntff_profile_hook = set_axon_ntff_profile_hook
        m.get_axon_ntff_profile_hook = get_axon_ntff_profile_hook
        sys.modules["antenv.axon_hooks"] = m
        antenv.axon_hooks = m
        try:
            from trn_agent_boot.trn_boot import _ntff_profile_via_ctypes

            hook = _ntff_profile_via_ctypes("/opt/axon/libaxon_pjrt.so")
            if hook is not None:
                m._hook = hook
        except Exception:
            pass

    import concourse.bass_utils as bu
    import concourse.bass2jax as b2j

    if not getattr(bu, "_drain_patch_installed", False):
        bu._drain_patch_installed = True
        bu.upload_artifacts = lambda tmpdir: "local://" + str(tmpdir)

        _orig = b2j.compile_bir_kernel

        def _patched_compile(ant_bir_str, compile_dir, neff_name="file.neff"):
            # This walrus build's codegen accepts at most ONE sync-wait per
            # instruction; hoist extras onto chained same-engine NoOps.
            d = json.loads(ant_bir_str)
            changed = False
            for fn in d.get("functions", []):
                for blk in fn.get("blocks", []):
                    insts = blk.get("instructions", [])
                    out = []
                    for ins in insts:
                        si = ins.get("sync_info") or {}
                        waits = si.get("on_wait") or []
                        if len(waits) > 1:
                            for ci, w in enumerate(waits[:-1]):
                                out.append(
                                    {
                                        "debug": ins.get("debug", 0),
                                        "engine": ins["engine"],
                                        "ins": [],
                                        "outs": [],
                                        "name": ins["name"] + f"-ws{ci}",
                                        "opcode": "NoOp",
                                        "sync_info": {
                                            "on_update": [],
                                            "on_wait": [w],
                                        },
                                    }
                                )
                            si["on_wait"] = waits[-1:]
                            changed = True
                        out.append(ins)
                    blk["instructions"] = out
            if changed:
                ant_bir_str = json.dumps(d).encode()
            return _orig(ant_bir_str, compile_dir, neff_name=neff_name)

        b2j.compile_bir_kernel = _patched_compile


_install_shims()

import concourse.bass as bass
import concourse.mybir as mybir
import concourse.tile as tile
from concourse.bass_utils import run_bass_kernel_spmd

f32 = mybir.dt.float32
bf16 = mybir.dt.bfloat16
AF = mybir.ActivationFunctionType
AX = mybir.AxisListType
ALU = mybir.AluOpType

# Problem constants (hardcoded per spec)
N_SEQ = 3137
N_SEQ_PAD = 3152
DIM = 512
H = 8
DH = 64
F = 16
NF = 196  # tokens per frame
NK = 197  # keys per frame block (frame + cls); also queries per block (+cls q)
PAIR = 2 * NF  # 392
PK = 2 * NK  # 394
N_CORES = 8

TOK_CHUNKS = [(0, 128), (128, 68)]


def build_kernel():
    nc = bass.Bass()
    x_d = nc.dram_tensor("x", [N_SEQ_PAD, DIM], bf16, kind="ExternalInput")
    wqkv_d = nc.dram_tensor("wqkv", [DIM, 3 * DIM], bf16, kind="ExternalInput")
    wout_d = nc.dram_tensor("wout", [DIM, DIM], bf16, kind="ExternalInput")
    bout_d = nc.dram_tensor("bout", [1, DIM], f32, kind="ExternalInput")
    ident_d = nc.dram_tensor("ident", [128, 128], bf16, kind="ExternalInput")
    ident32_d = nc.dram_tensor("ident32", [80, 65], f32, kind="ExternalInput")
    ind8_d = nc.dram_tensor("ind8", [8, DIM], bf16, kind="ExternalInput")
    qkTcls_d = nc.dram_tensor("qkTcls", [128, 8], bf16, kind="ExternalInput")
    vextcls_d = nc.dram_tensor("vextcls", [1, 8 * 65], bf16, kind="ExternalInput")
    corr8_d = nc.dram_tensor("corr8", [8, 65], f32, kind="ExternalInput")
    out_d = nc.dram_tensor("out", [N_SEQ, DIM], f32, kind="ExternalOutput")

    with tile.TileContext(nc) as tc:
        with (
            tc.tile_pool(name="const", bufs=1) as cpool,
            tc.tile_pool(name="work", bufs=2) as wpool,
            tc.tile_pool(name="at", bufs=4) as apool,
            tc.tile_pool(name="big_ps", bufs=2, space="PSUM") as big_ps,
            tc.tile_pool(name="attn_ps", bufs=3, space="PSUM") as attn_ps,
            tc.tile_pool(name="po_ps", bufs=2, space="PSUM") as po_ps,
            tc.tile_pool(name="rsb_ps", bufs=1, space="PSUM") as rsb_ps,
        ):
            # ---------------- preamble: DMAs in latency order ----------------
            ident = cpool.tile([128, 128], bf16, name="ident", tag="ident")
            nc.sync.dma_start(out=ident[:], in_=ident_d[:])
            # tiny cls constants first: frame 0's scores/po need them early
            qkTcls = cpool.tile([128, 8], bf16, name="qkTcls", tag="qkTcls")
            nc.sync.dma_start(out=qkTcls[:], in_=qkTcls_d[:])
            vextcls = cpool.tile([1, 8 * 65], bf16, name="vextcls", tag="vextcls")
            nc.sync.dma_start(out=vextcls[:], in_=vextcls_d[:])

            def stage_load(fp):
                """DMA the 2*NF x rows of pair fp (already bf16 in DRAM)."""
                xbf = []
                pr0 = 1 + fp * PAIR
                for fl in range(2):
                    for t, (t0, tn) in enumerate(TOK_CHUNKS):
                        i = 2 * fl + t
                        pt = 128 if t == 0 else 80  # pad rows to /16 for xbar
                        xb = wpool.tile(
                            [pt, DIM], bf16, name=f"xbf_{i}", tag=f"xbf_{i}"
                        )
                        if t == 1:
                            nc.gpsimd.memset(xb[64:80, :], 0.0)
                        nc.sync.dma_start(
                            out=xb[0:tn, :],
                            in_=x_d[pr0 + fl * NF + t0 : pr0 + fl * NF + t0 + tn, :],
                        )
                        xbf.append(xb)
                return xbf

            xbf_cur = stage_load(0)

            # v_ext ring: 2 frames in flight; ones cols + cls row written once
            # (early: frame 0's po matmuls need the cls row).
            vext_ring = []
            for r in range(2):
                pair_t = []
                for t, (t0, tn) in enumerate(TOK_CHUNKS):
                    pn = 128 if t == 0 else 69
                    vx = cpool.tile(
                        [pn, 8 * 65], bf16, name=f"vext_r{r}_{t}", tag=f"vext_r{r}_{t}"
                    )
                    nc.gpsimd.memset(
                        vx[:].rearrange("p (h c) -> p h c", c=65)[:, :, 64:65], 1.0
                    )
                    if t == 1:
                        nc.sync.dma_start(out=vx[68:69, :], in_=vextcls[:])
                    pair_t.append(vx)
                vext_ring.append(pair_t)

            # weight columns in consumption order: q, then k, then v — the qk
            # projection can start as soon as the first two blocks land.
            wqkv_bf = [
                cpool.tile([128, 3 * DIM], bf16, name=f"wqkv{c}", tag=f"wqkv{c}")
                for c in range(4)
            ]
            for blk in range(3):
                for c in range(4):
                    nc.sync.dma_start(
                        out=wqkv_bf[c][:, blk * DIM : (blk + 1) * DIM],
                        in_=wqkv_d[c * 128 : (c + 1) * 128, blk * DIM : (blk + 1) * DIM],
                    )

            def stage_transpose(fp, xbf):
                xTs = []
                for c in range(4):
                    ps_t = attn_ps.tile([128, PAIR], bf16, name="ps_t", tag="attn")
                    for fl in range(2):
                        for t, (t0, tn) in enumerate(TOK_CHUNKS):
                            g0 = fl * NF + t0
                            nc.tensor.transpose(
                                ps_t[:, g0 : g0 + tn],
                                xbf[2 * fl + t][0:tn, c * 128 : (c + 1) * 128],
                                ident[0:tn, 0:tn],
                            )
                    xt = wpool.tile([128, PAIR], bf16, name=f"xT_{c}", tag=f"xT_{c}")
                    nc.vector.tensor_copy(xt[:], ps_t[:])
                    xTs.append(xt)
                return xTs

            xT_cur = stage_transpose(0, xbf_cur)

            wout_bf = []
            for c in range(4):
                tb = cpool.tile([128, DIM], bf16, name=f"wout{c}", tag=f"wout{c}")
                nc.sync.dma_start(out=tb[:], in_=wout_d[c * 128 : (c + 1) * 128, :])
                wout_bf.append(tb)

            # small constants
            corr8 = cpool.tile([8, 65], f32, name="corr8", tag="corr8")
            nc.sync.dma_start(out=corr8[:], in_=corr8_d[:])
            ident32 = cpool.tile([80, 65], f32, name="ident32", tag="ident32")
            nc.sync.dma_start(out=ident32[:], in_=ident32_d[:])
            ind8 = cpool.tile([8, DIM], bf16, name="ind8", tag="ind8")
            nc.sync.dma_start(out=ind8[:], in_=ind8_d[:])
            bout_sb = cpool.tile([1, DIM], f32, name="bout", tag="bout")
            nc.sync.dma_start(out=bout_sb[:], in_=bout_d[:])

            ones_row = cpool.tile([1, 128], f32, name="ones_row", tag="ones_row")
            nc.gpsimd.memset(ones_row[:], 1.0)
            neg1 = cpool.tile([8, 1], f32, name="neg1", tag="neg1")
            nc.gpsimd.memset(neg1[:], -1.0)

            ps_b = big_ps.tile([128, DIM], f32, name="big", tag="big")
            nc.tensor.matmul(
                ps_b[:], lhsT=ones_row[:], rhs=bout_sb[:], start=True, stop=True
            )
            bout_bc = cpool.tile([128, DIM], f32, name="bout_bc", tag="bout_bc")
            nc.vector.tensor_copy(bout_bc[:], ps_b[:])

            # ---------------- static ring tiles ----------------
            # kq ring: [128, PK] bf16 per m-chunk; cols 196/393 hold the cls
            # q (m<4) / k (m>=4) vector, written once here.
            kq_ring = []
            for r in range(3):
                ring = []
                for m in range(8):
                    t = cpool.tile(
                        [128, PK], bf16, name=f"kq_r{r}_m{m}", tag=f"kq_r{r}_m{m}"
                    )
                    nc.scalar.copy(t[:, NF : NF + 1], qkTcls[:, m : m + 1])
                    nc.scalar.copy(t[:, NK + NF : NK + NF + 1], qkTcls[:, m : m + 1])
                    ring.append(t)
                kq_ring.append(ring)

            # cls accumulation columns: col h*F+fi = head-h contribution of
            # frame fi ([num(64) | den] on partitions 0..64).
            cls_cols = cpool.tile([65, H * F], f32, name="cls_cols", tag="cls_cols")

            # ---------------- stage functions ----------------
            def stage_qk(fp, xT):
                kq = kq_ring[fp % 3]
                for m in range(8):
                    ps_p = attn_ps.tile([128, PAIR], f32, name="ps_p", tag="attn")
                    for c in range(4):
                        nc.tensor.matmul(
                            ps_p[:],
                            lhsT=wqkv_bf[c][:, m * 128 : (m + 1) * 128],
                            rhs=xT[c][:, 0:PAIR],
                            start=(c == 0),
                            stop=(c == 3),
                        )
                    nc.vector.tensor_copy(
                        kq[m][:, 0:PK].rearrange("p (f k) -> p f k", k=NK)[:, :, 0:NF],
                        ps_p[:, 0:PAIR].rearrange("p (f k) -> p f k", k=NF),
                    )
                return kq

            def stage_v(fi, xT):
                fl = fi % 2
                xbase = fl * NF
                vext = vext_ring[fi % 2]
                for t, (t0, tn) in enumerate(TOK_CHUNKS):
                    ps_v = big_ps.tile([tn, DIM], f32, name="ps_v", tag="big")
                    for c in range(4):
                        nc.tensor.matmul(
                            ps_v[:],
                            lhsT=xT[c][:, xbase + t0 : xbase + t0 + tn],
                            rhs=wqkv_bf[c][:, 2 * DIM : 3 * DIM],
                            start=(c == 0),
                            stop=(c == 3),
                        )
                    nc.vector.tensor_copy(
                        vext[t][0:tn, :].rearrange("p (h c) -> p h c", c=65)[
                            :, :, 0:64
                        ],
                        ps_v[:].rearrange("p (h c) -> p h c", c=64),
                    )
                return vext

            def attn_head(fi, h, kq, vext, s8, attnT):
                fl = fi % 2
                kbase = fl * NK
                hc = h // 2
                r = (h % 2) * 64
                kT = kq[4 + hc]
                qT = kq[hc]
                ps_s = attn_ps.tile([128, PK], f32, name="ps_s", tag="attn")
                nc.tensor.matmul(
                    ps_s[:, 0:NK],
                    lhsT=kT[r : r + 64, kbase : kbase + 128],
                    rhs=qT[r : r + 64, kbase : kbase + NK],
                    start=True,
                    stop=True,
                )
                nc.tensor.matmul(
                    ps_s[0:69, NK:PK],
                    lhsT=kT[r : r + 64, kbase + 128 : kbase + NK],
                    rhs=qT[r : r + 64, kbase : kbase + NK],
                    start=True,
                    stop=True,
                )
                aT = apool.tile([128, PK], bf16, name="aT", tag="aT")
                nc.scalar.activation(aT[:], ps_s[:], AF.Exp)
                po = po_ps.tile([65, NK], f32, name="po", tag="po")
                nc.tensor.matmul(
                    po[:],
                    lhsT=vext[0][:, h * 65 : (h + 1) * 65],
                    rhs=aT[:, 0:NK],
                    start=True,
                    stop=False,
                )
                nc.tensor.matmul(
                    po[:],
                    lhsT=vext[1][0:69, h * 65 : (h + 1) * 65],
                    rhs=aT[0:69, NK:PK],
                    start=False,
                    stop=True,
                )
                nc.vector.tensor_copy(attnT[hc][r : r + 64, :], po[0:64, 0:NF])
                nc.scalar.activation(
                    s8[0:1, h * PAIR + fl * NF : h * PAIR + fl * NF + NF],
                    po[64:65, 0:NF],
                    AF.Ln,
                )
                nc.vector.tensor_copy(
                    cls_cols[0:65, h * F + fi : h * F + fi + 1], po[0:65, NF : NF + 1]
                )

            def recip_exp(s8):
                # s8 row holds ln(den) per (head, frame-of-pair); reshape to
                # [8, PAIR] via DMA, then rs8 = exp(-.) = 1/den in bf16
                s8p = wpool.tile([8, PAIR], f32, name="s8p", tag="s8p")
                nc.sync.dma_start(out=s8p[:], in_=s8[0:1, :])
                rs8 = wpool.tile([8, PAIR], bf16, name="rs8", tag="rs8")
                nc.scalar.activation(rs8[:], s8p[:], AF.Exp, scale=neg1[:])
                return rs8

            def stage_norm(rs8, attnT_a, attnT_b, c):
                ps_r = rsb_ps.tile([128, PAIR], f32, name="ps_r", tag="rsb")
                nc.tensor.matmul(
                    ps_r[:],
                    lhsT=ind8[:, c * 128 : (c + 1) * 128],
                    rhs=rs8[:],
                    start=True,
                    stop=True,
                )
                nc.vector.tensor_mul(attnT_a[c][:], attnT_a[c][:], ps_r[:, 0:NF])
                nc.vector.tensor_mul(attnT_b[c][:], attnT_b[c][:], ps_r[:, NF:PAIR])

            def stage_norm_half(rs8, attnT, c, half):
                # normalize one frame of a pair (used to drain the last pair)
                ps_r = rsb_ps.tile([128, PAIR], f32, name="ps_r", tag="rsb")
                nc.tensor.matmul(
                    ps_r[:, 0:NF],
                    lhsT=ind8[:, c * 128 : (c + 1) * 128],
                    rhs=rs8[:, half * NF : half * NF + NF],
                    start=True,
                    stop=True,
                )
                nc.vector.tensor_mul(attnT[c][:], attnT[c][:], ps_r[:, 0:NF])

            def stage_out(fi, t, attnT):
                r0 = 1 + fi * NF
                t0, tn = TOK_CHUNKS[t]
                ps_o = big_ps.tile([tn, DIM], f32, name="ps_o", tag="big")
                for c in range(4):
                    nc.tensor.matmul(
                        ps_o[:],
                        lhsT=attnT[c][:, t0 : t0 + tn],
                        rhs=wout_bf[c][:],
                        start=(c == 0),
                        stop=(c == 3),
                    )
                o_sb = wpool.tile([tn, DIM], f32, name=f"osb_{t}", tag=f"osb_{t}")
                nc.vector.tensor_add(o_sb[:], ps_o[:], bout_bc[0:tn, :])
                nc.sync.dma_start(out=out_d[r0 + t0 : r0 + t0 + tn, :], in_=o_sb[:])

            # pair-0 qk projection (needs wqkv + pair-0 transposes)
            kq_cur = stage_qk(0, xT_cur)

            # ---------------- main loop: 16 frames, 1-pair pipeline ---------
            pending = None  # (pair, s8row, attnT of frame a, attnT of frame b)
            xbf_next = None
            xT_next = None
            kq_next = None
            s8_pair = None
            attnT_a = None
            prs8 = None
            for fi in range(F):
                fp, fl = fi // 2, fi % 2
                if fl == 0 and fp + 1 < F // 2:
                    xbf_next = stage_load(fp + 1)
                vext = stage_v(fi, xT_cur)
                if fl == 0:
                    s8_pair = wpool.tile([1, H * PAIR], f32, name="s8", tag="s8")
                attnT = [
                    wpool.tile(
                        [128, NF], bf16, name=f"attnT_{c}", tag=f"attnT_{c}", bufs=4
                    )
                    for c in range(4)
                ]
                for h in range(H):
                    attn_head(fi, h, kq_cur, vext, s8_pair, attnT)
                    if pending is not None:
                        pa, pb, ps8 = pending
                        if fl == 0:
                            if h == 0:
                                prs8 = recip_exp(ps8)
                            elif 1 <= h <= 4:
                                stage_norm(prs8, pa, pb, h - 1)
                            elif h == 5:
                                stage_out(2 * fp - 2, 0, pa)
                            elif h == 6:
                                stage_out(2 * fp - 2, 1, pa)
                            elif h == 7:
                                stage_out(2 * fp - 1, 0, pb)
                        else:
                            if h == 0:
                                stage_out(2 * fp - 1, 1, pb)
                    if fl == 1 and fp + 1 < F // 2:
                        if h == 5:
                            xT_next = stage_transpose(fp + 1, xbf_next)
                        elif h == 6:
                            kq_next = stage_qk(fp + 1, xT_next)
                    if fl == 1 and fp == F // 2 - 1:
                        # last pair: drain frame 14's norm/out inside frame
                        # 15's head loop (its half of the dens is complete)
                        if h == 1:
                            prs8_a = recip_exp(s8_pair)
                        elif h in (2, 3):
                            stage_norm_half(prs8_a, attnT_a, 2 * (h - 2), 0)
                            stage_norm_half(prs8_a, attnT_a, 2 * (h - 2) + 1, 0)
                        elif h == 4:
                            stage_out(2 * fp, 0, attnT_a)
                        elif h == 5:
                            stage_out(2 * fp, 1, attnT_a)
                if fl == 0:
                    attnT_a = attnT
                else:
                    pending = (attnT_a, attnT, s8_pair)
                    if fp + 1 < F // 2:
                        xT_cur, kq_cur = xT_next, kq_next

            # drain: frame 15's norm + out-proj, interleaved with the cls
            # epilogue's serial Vector/PE chain so the PE stays fed.
            pa, pb, ps8 = pending
            prs8 = recip_exp(ps8)
            red = wpool.tile([65, 8], f32, name="red", tag="red")
            nc.vector.tensor_reduce(
                red[:],
                cls_cols[:].rearrange("p (h f) -> p h f", f=F),
                axis=AX.X,
                op=ALU.add,
            )
            stage_norm_half(prs8, pb, 0, 1)
            stage_norm_half(prs8, pb, 1, 1)
            ps_tr = po_ps.tile([8, 65], f32, name="ps_tr", tag="po")
            nc.tensor.transpose(ps_tr[:], red[:], ident32[0:65, :])
            stage_norm_half(prs8, pb, 2, 1)
            stage_norm_half(prs8, pb, 3, 1)
            acc = wpool.tile([8, 65], f32, name="acc", tag="acc")
            nc.vector.tensor_sub(acc[:], ps_tr[:], corr8[:])
            rden = wpool.tile([8, 1], f32, name="rden", tag="rden")
            nc.vector.reciprocal(rden[:], acc[:, 64:65])
            cls_n = wpool.tile([8, 64], bf16, name="cls_n", tag="cls_n")
            nc.vector.tensor_scalar_mul(cls_n[:], acc[:, 0:64], rden[:, 0:1])
            stage_out(15, 0, pb)
            ps_t2 = attn_ps.tile([64, 8], bf16, name="ps_t2", tag="attn")
            nc.tensor.transpose(ps_t2[:], cls_n[:], ident[0:8, 0:8])
            attnT_cls = [
                wpool.tile([128, 1], bf16, name=f"aTc{c}", tag=f"aTc{c}")
                for c in range(4)
            ]
            for h in range(8):
                nc.vector.tensor_copy(
                    attnT_cls[h // 2][(h % 2) * 64 : (h % 2) * 64 + 64, :],
                    ps_t2[:, h : h + 1],
                )
            stage_out(15, 1, pb)
            ps_oc = big_ps.tile([1, DIM], f32, name="ps_oc", tag="big")
            for c in range(4):
                nc.tensor.matmul(
                    ps_oc[:],
                    lhsT=attnT_cls[c][:],
                    rhs=wout_bf[c][:],
                    start=(c == 0),
                    stop=(c == 3),
                )
            o_cls = wpool.tile([1, DIM], f32, name="o_cls", tag="o_cls")
            nc.vector.tensor_add(o_cls[:], ps_oc[:], bout_bc[0:1, :])
            nc.sync.dma_start(out=out_d[0:1, :], in_=o_cls[:])

    return nc


_NC_CACHE = {}


def _get_nc():
    if "nc" not in _NC_CACHE:
        _NC_CACHE["nc"] = build_kernel()
    return _NC_CACHE["nc"]


def kernel(x, Wqkv, Wout, bout, f, _trace=False, _trace_kwargs=None):
    assert int(f) == F, f"kernel hardcoded for f={F}, got {f}"
    import ml_dtypes

    bf = ml_dtypes.bfloat16
    x = np.asarray(x, np.float32)
    x_pad = np.zeros((N_CORES, N_SEQ_PAD, DIM), dtype=bf)
    x_pad[:, :N_SEQ] = x.astype(bf)
    Wqkv_s = np.asarray(Wqkv, np.float32).copy()
    Wqkv_s[:, :DIM] *= DH ** -0.5  # fold q scaling into the projection
    Wqkv_bf = Wqkv_s.astype(bf)
    Wout_bf = np.asarray(Wout, np.float32).astype(bf)
    bout2 = np.asarray(bout, np.float32).reshape(1, DIM)

    ident_np = np.eye(128, dtype=bf)
    ident32_np = np.zeros((80, 65), dtype=np.float32)
    ident32_np[:65] = np.eye(65, dtype=np.float32)
    ind8_np = np.zeros((8, DIM), dtype=bf)
    for k in range(8):
        ind8_np[k, k * 64 : (k + 1) * 64] = 1.0

    # host-precomputed cls constants (mirrors the device bf16 dataflow)
    nc = _get_nc()
    in_maps = []
    for i in range(N_CORES):
        xcls_bf = x_pad[i, 0, :].astype(np.float32)  # bf16-quantized cls row
        qkv_cls = xcls_bf @ Wqkv_bf.astype(np.float32)  # [1536] fp32
        qkT_np = np.zeros((128, 8), dtype=bf)
        for m in range(8):
            qkT_np[:, m] = qkv_cls[m * 128 : (m + 1) * 128].astype(bf)
        v_cls_bf = qkv_cls[2 * DIM : 3 * DIM].astype(bf).astype(np.float32)
        vext_np = np.zeros((1, 8 * 65), dtype=bf)
        for hh in range(8):
            vext_np[0, hh * 65 : hh * 65 + 64] = v_cls_bf[hh * 64 : (hh + 1) * 64]
            vext_np[0, hh * 65 + 64] = 1.0
        # self-score from the bf16-quantized q/k (matches frame matmuls)
        qb = qkT_np[:, 0:4].astype(np.float32).T.reshape(DIM)
        kb = qkT_np[:, 4:8].astype(np.float32).T.reshape(DIM)
        s_self = (qb.reshape(8, 64) * kb.reshape(8, 64)).sum(axis=1)  # [8]
        a15 = 15.0 * np.exp(s_self)
        corr_np = np.zeros((8, 65), dtype=np.float32)
        corr_np[:, 0:64] = a15[:, None] * v_cls_bf.reshape(8, 64)
        corr_np[:, 64] = a15
        in_maps.append(
            {
                "x": x_pad[i],
                "wqkv": Wqkv_bf,
                "wout": Wout_bf,
                "bout": bout2,
                "ident": ident_np,
                "ident32": ident32_np,
                "ind8": ind8_np,
                "qkTcls": qkT_np,
                "vextcls": vext_np,
                "corr8": corr_np,
            }
        )
    res = run_bass_kernel_spmd(
        nc,
        in_maps,
        list(range(N_CORES)),
        trace=_trace,
        **(_trace_kwargs or {}),
    )
    out = np.stack([res.results[i]["out"] for i in range(N_CORES)], axis=0)
    if _trace:
        kernel.last_results = res
    return out


# revision 62
# speedup vs baseline: 1.0155x; 1.0110x over previous
"""Trainium2 Bass kernel for nn_Attention_29935922053658 (sparse frame attention).

Sharding: data-parallel over batch B=8 -> 8 NeuronCores (1 batch each).
Per-core: fused qkv-proj + frame-local attention (196-token frames, cls token
attends globally) + out-proj, streamed per frame with bf16 matmuls / fp32 accum.

v5: cls attention piggybacks on the frame score/AV matmuls (cls-q as a 197th
query column; 15x self-term overcount corrected at the epilogue with
host-precomputed constants), denominator reciprocals via a fused Ln/Exp(-x)
pair, pair-level normalization, and a 1-pair software pipeline so the PE never
drains. x and the weights are pre-cast to bf16 on the host.
"""

import sys
import types
import json
import math

for _p in ("/opt/trn_rl_repo", "/root/.axon_site"):
    if _p not in sys.path:
        sys.path.insert(0, _p)

import numpy as np

# ---------------------------------------------------------------------------
# Environment shims (required under the axon-proxied PJRT runtime):
#  1. antenv.axon_hooks registry (missing in this image) so trace=True can work.
#  2. Split >2 sync-waits off Drain instructions — this walrus build's CoreV3
#     codegen rejects them ("Too many sync wait commands").
#  3. upload_artifacts: no artifact bucket in this container.
# ---------------------------------------------------------------------------


def _install_shims():
    import antenv

    if "antenv.axon_hooks" not in sys.modules:
        m = types.ModuleType("antenv.axon_hooks")
        m._hook = None

        def set_axon_ntff_profile_hook(h):
            m._hook = h

        def get_axon_ntff_profile_hook():
            return m._hook

        m.set_axon_ntff_profile_hook = set_axon_ntff_profile_hook
        m.get_axon_ntff_profile_hook = get_axon_ntff_profile_hook
        sys.modules["antenv.axon_hooks"] = m
        antenv.axon_hooks = m
        try:
            from trn_agent_boot.trn_boot import _ntff_profile_via_ctypes

            hook = _ntff_profile_via_ctypes("/opt/axon/libaxon_pjrt.so")
            if hook is not None:
                m._hook = hook
        except Exception:
            pass

    import concourse.bass_utils as bu
    import concourse.bass2jax as b2j

    if not getattr(bu, "_drain_patch_installed", False):
        bu._drain_patch_installed = True
        bu.upload_artifacts = lambda tmpdir: "local://" + str(tmpdir)

        _orig = b2j.compile_bir_kernel

        def _patched_compile(ant_bir_str, compile_dir, neff_name="file.neff"):
            # This walrus build's codegen accepts at most ONE sync-wait per
            # instruction; hoist extras onto chained same-engine NoOps.
            d = json.loads(ant_bir_str)
            changed = False
            for fn in d.get("functions", []):
                for blk in fn.get("blocks", []):
                    insts = blk.get("instructions", [])
                    out = []
                    for ins in insts:
                        si = ins.get("sync_info") or {}
                        waits = si.get("on_wait") or []
                        if len(waits) > 1:
                            for ci, w in enumerate(waits[:-1]):
                                out.append(
                                    {
                                        "debug": ins.get("debug", 0),
                                        "engine": ins["engine"],
                                        "ins": [],
                                        "outs": [],
                                        "name": ins["name"] + f"-ws{ci}",
                                        "opcode": "NoOp",
                                        "sync_info": {
                                            "on_update": [],
                                            "on_wait": [w],
                                        },
                                    }
                                )
                            si["on_wait"] = waits[-1:]
                            changed = True
                        out.append(ins)
                    blk["instructions"] = out
            if changed:
                ant_bir_str = json.dumps(d).encode()
            return _orig(ant_bir_str, compile_dir, neff_name=neff_name)

        b2j.compile_bir_kernel = _patched_compile


_install_shims()

import concourse.bass as bass
import concourse.mybir as mybir
import concourse.tile as tile
from concourse.bass_utils import run_bass_kernel_spmd

f32 = mybir.dt.float32
bf16 = mybir.dt.bfloat16
AF = mybir.ActivationFunctionType
AX = mybir.AxisListType
ALU = mybir.AluOpType

# Problem constants (hardcoded per spec)
N_SEQ = 3137
N_SEQ_PAD = 3152
DIM = 512
H = 8
DH = 64
F = 16
NF = 196  # tokens per frame
NK = 197  # keys per frame block (frame + cls); also queries per block (+cls q)
PAIR = 2 * NF  # 392
PK = 2 * NK  # 394
N_CORES = 8

TOK_CHUNKS = [(0, 128), (128, 68)]


def build_kernel():
    nc = bass.Bass()
    x_d = nc.dram_tensor("x", [N_SEQ_PAD, DIM], bf16, kind="ExternalInput")
    wqkv_d = nc.dram_tensor("wqkv", [DIM, 3 * DIM], bf16, kind="ExternalInput")
    wout_d = nc.dram_tensor("wout", [DIM, DIM], bf16, kind="ExternalInput")
    bout_d = nc.dram_tensor("bout", [1, DIM], f32, kind="ExternalInput")
    ident_d = nc.dram_tensor("ident", [128, 128], bf16, kind="ExternalInput")
    ident32_d = nc.dram_tensor("ident32", [80, 65], f32, kind="ExternalInput")
    ind8_d = nc.dram_tensor("ind8", [8, DIM], bf16, kind="ExternalInput")
    qkTcls_d = nc.dram_tensor("qkTcls", [128, 8], bf16, kind="ExternalInput")
    vextcls_d = nc.dram_tensor("vextcls", [1, 8 * 65], bf16, kind="ExternalInput")
    corr8_d = nc.dram_tensor("corr8", [8, 65], f32, kind="ExternalInput")
    out_d = nc.dram_tensor("out", [N_SEQ, DIM], f32, kind="ExternalOutput")

    with tile.TileContext(nc) as tc:
        with (
            tc.tile_pool(name="const", bufs=1) as cpool,
            tc.tile_pool(name="work", bufs=2) as wpool,
            tc.tile_pool(name="at", bufs=4) as apool,
            tc.tile_pool(name="big_ps", bufs=2, space="PSUM") as big_ps,
            tc.tile_pool(name="attn_ps", bufs=3, space="PSUM") as attn_ps,
            tc.tile_pool(name="po_ps", bufs=2, space="PSUM") as po_ps,
            tc.tile_pool(name="rsb_ps", bufs=1, space="PSUM") as rsb_ps,
        ):
            # ---------------- preamble: DMAs in latency order ----------------
            ident = cpool.tile([128, 128], bf16, name="ident", tag="ident")
            nc.sync.dma_start(out=ident[:], in_=ident_d[:])
            # tiny cls constants first: frame 0's scores/po need them early
            qkTcls = cpool.tile([128, 8], bf16, name="qkTcls", tag="qkTcls")
            nc.sync.dma_start(out=qkTcls[:], in_=qkTcls_d[:])
            vextcls = cpool.tile([1, 8 * 65], bf16, name="vextcls", tag="vextcls")
            nc.sync.dma_start(out=vextcls[:], in_=vextcls_d[:])

            def stage_load(fp):
                """DMA the 2*NF x rows of pair fp (already bf16 in DRAM)."""
                xbf = []
                pr0 = 1 + fp * PAIR
                for fl in range(2):
                    for t, (t0, tn) in enumerate(TOK_CHUNKS):
                        i = 2 * fl + t
                        pt = 128 if t == 0 else 80  # pad rows to /16 for xbar
                        xb = wpool.tile(
                            [pt, DIM], bf16, name=f"xbf_{i}", tag=f"xbf_{i}"
                        )
                        if t == 1:
                            nc.gpsimd.memset(xb[64:80, :], 0.0)
                        nc.sync.dma_start(
                            out=xb[0:tn, :],
                            in_=x_d[pr0 + fl * NF + t0 : pr0 + fl * NF + t0 + tn, :],
                        )
                        xbf.append(xb)
                return xbf

            xbf_cur = stage_load(0)

            # v_ext ring: 2 frames in flight; ones cols + cls row written once
            # (early: frame 0's po matmuls need the cls row).
            vext_ring = []
            for r in range(2):
                pair_t = []
                for t, (t0, tn) in enumerate(TOK_CHUNKS):
                    pn = 128 if t == 0 else 69
                    vx = cpool.tile(
                        [pn, 8 * 65], bf16, name=f"vext_r{r}_{t}", tag=f"vext_r{r}_{t}"
                    )
                    nc.gpsimd.memset(
                        vx[:].rearrange("p (h c) -> p h c", c=65)[:, :, 64:65], 1.0
                    )
                    if t == 1:
                        nc.sync.dma_start(out=vx[68:69, :], in_=vextcls[:])
                    pair_t.append(vx)
                vext_ring.append(pair_t)

            # weight columns in consumption order: q, then k, then v — the qk
            # projection can start as soon as the first two blocks land.
            wqkv_bf = [
                cpool.tile([128, 3 * DIM], bf16, name=f"wqkv{c}", tag=f"wqkv{c}")
                for c in range(4)
            ]
            for blk in range(3):
                for c in range(4):
                    nc.sync.dma_start(
                        out=wqkv_bf[c][:, blk * DIM : (blk + 1) * DIM],
                        in_=wqkv_d[c * 128 : (c + 1) * 128, blk * DIM : (blk + 1) * DIM],
                    )

            def stage_transpose(fp, xbf):
                xTs = []
                for c in range(4):
                    ps_t = attn_ps.tile([128, PAIR], bf16, name="ps_t", tag="attn")
                    for fl in range(2):
                        for t, (t0, tn) in enumerate(TOK_CHUNKS):
                            g0 = fl * NF + t0
                            nc.tensor.transpose(
                                ps_t[:, g0 : g0 + tn],
                                xbf[2 * fl + t][0:tn, c * 128 : (c + 1) * 128],
                                ident[0:tn, 0:tn],
                            )
                    xt = wpool.tile([128, PAIR], bf16, name=f"xT_{c}", tag=f"xT_{c}")
                    nc.vector.tensor_copy(xt[:], ps_t[:])
                    xTs.append(xt)
                return xTs

            xT_cur = stage_transpose(0, xbf_cur)

            wout_bf = []
            for c in range(4):
                tb = cpool.tile([128, DIM], bf16, name=f"wout{c}", tag=f"wout{c}")
                nc.sync.dma_start(out=tb[:], in_=wout_d[c * 128 : (c + 1) * 128, :])
                wout_bf.append(tb)

            # small constants
            corr8 = cpool.tile([8, 65], f32, name="corr8", tag="corr8")
            nc.sync.dma_start(out=corr8[:], in_=corr8_d[:])
            ident32 = cpool.tile([80, 65], f32, name="ident32", tag="ident32")
            nc.sync.dma_start(out=ident32[:], in_=ident32_d[:])
            ind8 = cpool.tile([8, DIM], bf16, name="ind8", tag="ind8")
            nc.sync.dma_start(out=ind8[:], in_=ind8_d[:])
            bout_sb = cpool.tile([1, DIM], f32, name="bout", tag="bout")
            nc.sync.dma_start(out=bout_sb[:], in_=bout_d[:])

            ones_row = cpool.tile([1, 128], f32, name="ones_row", tag="ones_row")
            nc.gpsimd.memset(ones_row[:], 1.0)
            neg1 = cpool.tile([8, 1], f32, name="neg1", tag="neg1")
            nc.gpsimd.memset(neg1[:], -1.0)

            ps_b = big_ps.tile([128, DIM], f32, name="big", tag="big")
            nc.tensor.matmul(
                ps_b[:], lhsT=ones_row[:], rhs=bout_sb[:], start=True, stop=True
            )
            bout_bc = cpool.tile([128, DIM], f32, name="bout_bc", tag="bout_bc")
            nc.vector.tensor_copy(bout_bc[:], ps_b[:])

            # ---------------- static ring tiles ----------------
            # kq ring: [128, PK] bf16 per m-chunk; cols 196/393 hold the cls
            # q (m<4) / k (m>=4) vector, written once here.
            kq_ring = []
            for r in range(3):
                ring = []
                for m in range(8):
                    t = cpool.tile(
                        [128, PK], bf16, name=f"kq_r{r}_m{m}", tag=f"kq_r{r}_m{m}"
                    )
                    nc.scalar.copy(t[:, NF : NF + 1], qkTcls[:, m : m + 1])
                    nc.scalar.copy(t[:, NK + NF : NK + NF + 1], qkTcls[:, m : m + 1])
                    ring.append(t)
                kq_ring.append(ring)

            # cls accumulation columns: col h*F+fi = head-h contribution of
            # frame fi ([num(64) | den] on partitions 0..64).
            cls_cols = cpool.tile([65, H * F], f32, name="cls_cols", tag="cls_cols")

            # ---------------- stage functions ----------------
            def stage_qk(fp, xT):
                kq = kq_ring[fp % 3]
                for m in range(8):
                    ps_p = attn_ps.tile([128, PAIR], f32, name="ps_p", tag="attn")
                    for c in range(4):
                        nc.tensor.matmul(
                            ps_p[:],
                            lhsT=wqkv_bf[c][:, m * 128 : (m + 1) * 128],
                            rhs=xT[c][:, 0:PAIR],
                            start=(c == 0),
                            stop=(c == 3),
                        )
                    nc.vector.tensor_copy(
                        kq[m][:, 0:PK].rearrange("p (f k) -> p f k", k=NK)[:, :, 0:NF],
                        ps_p[:, 0:PAIR].rearrange("p (f k) -> p f k", k=NF),
                    )
                return kq

            def stage_v(fi, xT):
                fl = fi % 2
                xbase = fl * NF
                vext = vext_ring[fi % 2]
                for t, (t0, tn) in enumerate(TOK_CHUNKS):
                    ps_v = big_ps.tile([tn, DIM], f32, name="ps_v", tag="big")
                    for c in range(4):
                        nc.tensor.matmul(
                            ps_v[:],
                            lhsT=xT[c][:, xbase + t0 : xbase + t0 + tn],
                            rhs=wqkv_bf[c][:, 2 * DIM : 3 * DIM],
                            start=(c == 0),
                            stop=(c == 3),
                        )
                    nc.vector.tensor_copy(
                        vext[t][0:tn, :].rearrange("p (h c) -> p h c", c=65)[
                            :, :, 0:64
                        ],
                        ps_v[:].rearrange("p (h c) -> p h c", c=64),
                    )
                return vext

            def attn_head(fi, h, kq, vext, s8, attnT):
                fl = fi % 2
                kbase = fl * NK
                hc = h // 2
                r = (h % 2) * 64
                kT = kq[4 + hc]
                qT = kq[hc]
                ps_s = attn_ps.tile([128, PK], f32, name="ps_s", tag="attn")
                nc.tensor.matmul(
                    ps_s[:, 0:NK],
                    lhsT=kT[r : r + 64, kbase : kbase + 128],
                    rhs=qT[r : r + 64, kbase : kbase + NK],
                    start=True,
                    stop=True,
                )
                nc.tensor.matmul(
                    ps_s[0:69, NK:PK],
                    lhsT=kT[r : r + 64, kbase + 128 : kbase + NK],
                    rhs=qT[r : r + 64, kbase : kbase + NK],
                    start=True,
                    stop=True,
                )
                aT = apool.tile([128, PK], bf16, name="aT", tag="aT")
                nc.scalar.activation(aT[:], ps_s[:], AF.Exp)
                po = po_ps.tile([65, NK], f32, name="po", tag="po")
                nc.tensor.matmul(
                    po[:],
                    lhsT=vext[0][:, h * 65 : (h + 1) * 65],
                    rhs=aT[:, 0:NK],
                    start=True,
                    stop=False,
                )
                nc.tensor.matmul(
                    po[:],
                    lhsT=vext[1][0:69, h * 65 : (h + 1) * 65],
                    rhs=aT[0:69, NK:PK],
                    start=False,
                    stop=True,
                )
                nc.vector.tensor_copy(attnT[hc][r : r + 64, :], po[0:64, 0:NF])
                nc.scalar.activation(
                    s8[0:1, h * PAIR + fl * NF : h * PAIR + fl * NF + NF],
                    po[64:65, 0:NF],
                    AF.Ln,
                )
                nc.vector.tensor_copy(
                    cls_cols[0:65, h * F + fi : h * F + fi + 1], po[0:65, NF : NF + 1]
                )

            def recip_exp(s8):
                # s8 row holds ln(den) per (head, frame-of-pair); reshape to
                # [8, PAIR] via DMA, then rs8 = exp(-.) = 1/den in bf16
                s8p = wpool.tile([8, PAIR], f32, name="s8p", tag="s8p")
                nc.sync.dma_start(out=s8p[:], in_=s8[0:1, :])
                rs8 = wpool.tile([8, PAIR], bf16, name="rs8", tag="rs8")
                nc.scalar.activation(rs8[:], s8p[:], AF.Exp, scale=neg1[:])
                return rs8

            def stage_norm(rs8, attnT_a, attnT_b, c):
                ps_r = rsb_ps.tile([128, PAIR], f32, name="ps_r", tag="rsb")
                nc.tensor.matmul(
                    ps_r[:],
                    lhsT=ind8[:, c * 128 : (c + 1) * 128],
                    rhs=rs8[:],
                    start=True,
                    stop=True,
                )
                nc.vector.tensor_mul(attnT_a[c][:], attnT_a[c][:], ps_r[:, 0:NF])
                nc.vector.tensor_mul(attnT_b[c][:], attnT_b[c][:], ps_r[:, NF:PAIR])

            def stage_norm_half(rs8, attnT, c, half):
                # normalize one frame of a pair (used to drain the last pair)
                ps_r = rsb_ps.tile([128, PAIR], f32, name="ps_r", tag="rsb")
                nc.tensor.matmul(
                    ps_r[:, 0:NF],
                    lhsT=ind8[:, c * 128 : (c + 1) * 128],
                    rhs=rs8[:, half * NF : half * NF + NF],
                    start=True,
                    stop=True,
                )
                nc.vector.tensor_mul(attnT[c][:], attnT[c][:], ps_r[:, 0:NF])

            def stage_out(fi, t, attnT):
                r0 = 1 + fi * NF
                t0, tn = TOK_CHUNKS[t]
                ps_o = big_ps.tile([tn, DIM], f32, name="ps_o", tag="big")
                for c in range(4):
                    nc.tensor.matmul(
                        ps_o[:],
                        lhsT=attnT[c][:, t0 : t0 + tn],
                        rhs=wout_bf[c][:],
                        start=(c == 0),
                        stop=(c == 3),
                    )
                o_sb = wpool.tile([tn, DIM], f32, name=f"osb_{t}", tag=f"osb_{t}")
                nc.vector.tensor_add(o_sb[:], ps_o[:], bout_bc[0:tn, :])
                nc.sync.dma_start(out=out_d[r0 + t0 : r0 + t0 + tn, :], in_=o_sb[:])

            # pair-0 qk projection (needs wqkv + pair-0 transposes)
            kq_cur = stage_qk(0, xT_cur)

            # ---------------- main loop: 16 frames, 1-pair pipeline ---------
            pending = None  # (pair, s8row, attnT of frame a, attnT of frame b)
            xbf_next = None
            xT_next = None
            kq_next = None
            s8_pair = None
            attnT_a = None
            prs8 = None
            for fi in range(F):
                fp, fl = fi // 2, fi % 2
                if fl == 0 and fp + 1 < F // 2:
                    xbf_next = stage_load(fp + 1)
                vext = stage_v(fi, xT_cur)
                if fl == 0:
                    s8_pair = wpool.tile([1, H * PAIR], f32, name="s8", tag="s8")
                attnT = [
                    wpool.tile(
                        [128, NF], bf16, name=f"attnT_{c}", tag=f"attnT_{c}", bufs=4
                    )
                    for c in range(4)
                ]
                for h in range(H):
                    attn_head(fi, h, kq_cur, vext, s8_pair, attnT)
                    if pending is not None:
                        pa, pb, ps8 = pending
                        if fl == 0:
                            if h == 0:
                                prs8 = recip_exp(ps8)
                            elif 2 <= h <= 5:
                                stage_norm(prs8, pa, pb, h - 2)
                            elif h == 6:
                                stage_out(2 * fp - 2, 0, pa)
                            elif h == 7:
                                stage_out(2 * fp - 2, 1, pa)
                        else:
                            if h == 0:
                                stage_out(2 * fp - 1, 0, pb)
                            elif h == 1:
                                stage_out(2 * fp - 1, 1, pb)
                    if fl == 1 and fp + 1 < F // 2:
                        if h == 5:
                            xT_next = stage_transpose(fp + 1, xbf_next)
                        elif h == 6:
                            kq_next = stage_qk(fp + 1, xT_next)
                    if fl == 1 and fp == F // 2 - 1:
                        # last pair: drain frame 14's norm/out inside frame
                        # 15's head loop (its half of the dens is complete)
                        if h == 1:
                            prs8_a = recip_exp(s8_pair)
                        elif h in (2, 3):
                            stage_norm_half(prs8_a, attnT_a, 2 * (h - 2), 0)
                            stage_norm_half(prs8_a, attnT_a, 2 * (h - 2) + 1, 0)
                        elif h == 4:
                            stage_out(2 * fp, 0, attnT_a)
                        elif h == 5:
                            stage_out(2 * fp, 1, attnT_a)
                if fl == 0:
                    attnT_a = attnT
                else:
                    pending = (attnT_a, attnT, s8_pair)
                    if fp + 1 < F // 2:
                        xT_cur, kq_cur = xT_next, kq_next

            # drain: frame 15's norm + out-proj, interleaved with the cls
            # epilogue's serial Vector/PE chain so the PE stays fed.
            pa, pb, ps8 = pending
            prs8 = recip_exp(ps8)
            red = wpool.tile([65, 8], f32, name="red", tag="red")
            nc.vector.tensor_reduce(
                red[:],
                cls_cols[:].rearrange("p (h f) -> p h f", f=F),
                axis=AX.X,
                op=ALU.add,
            )
            stage_norm_half(prs8, pb, 0, 1)
            stage_norm_half(prs8, pb, 1, 1)
            ps_tr = po_ps.tile([8, 65], f32, name="ps_tr", tag="po")
            nc.tensor.transpose(ps_tr[:], red[:], ident32[0:65, :])
            stage_norm_half(prs8, pb, 2, 1)
            stage_norm_half(prs8, pb, 3, 1)
            acc = wpool.tile([8, 65], f32, name="acc", tag="acc")
            nc.vector.tensor_sub(acc[:], ps_tr[:], corr8[:])
            rden = wpool.tile([8, 1], f32, name="rden", tag="rden")
            nc.vector.reciprocal(rden[:], acc[:, 64:65])
            cls_n = wpool.tile([8, 64], bf16, name="cls_n", tag="cls_n")
            nc.vector.tensor_scalar_mul(cls_n[:], acc[:, 0:64], rden[:, 0:1])
            stage_out(15, 0, pb)
            ps_t2 = attn_ps.tile([64, 8], bf16, name="ps_t2", tag="attn")
            nc.tensor.transpose(ps_t2[:], cls_n[:], ident[0:8, 0:8])
            attnT_cls = [
                wpool.tile([128, 1], bf16, name=f"aTc{c}", tag=f"aTc{c}")
                for c in range(4)
            ]
            for h in range(8):
                nc.vector.tensor_copy(
                    attnT_cls[h // 2][(h % 2) * 64 : (h % 2) * 64 + 64, :],
                    ps_t2[:, h : h + 1],
                )
            stage_out(15, 1, pb)
            ps_oc = big_ps.tile([1, DIM], f32, name="ps_oc", tag="big")
            for c in range(4):
                nc.tensor.matmul(
                    ps_oc[:],
                    lhsT=attnT_cls[c][:],
                    rhs=wout_bf[c][:],
                    start=(c == 0),
                    stop=(c == 3),
                )
            o_cls = wpool.tile([1, DIM], f32, name="o_cls", tag="o_cls")
            nc.vector.tensor_add(o_cls[:], ps_oc[:], bout_bc[0:1, :])
            nc.sync.dma_start(out=out_d[0:1, :], in_=o_cls[:])

    return nc


_NC_CACHE = {}


def _get_nc():
    if "nc" not in _NC_CACHE:
        _NC_CACHE["nc"] = build_kernel()
    return _NC_CACHE["nc"]


def kernel(x, Wqkv, Wout, bout, f, _trace=False, _trace_kwargs=None):
    assert int(f) == F, f"kernel hardcoded for f={F}, got {f}"
    import ml_dtypes

    bf = ml_dtypes.bfloat16
    x = np.asarray(x, np.float32)
    x_pad = np.zeros((N_CORES, N_SEQ_PAD, DIM), dtype=bf)
    x_pad[:, :N_SEQ] = x.astype(bf)
    Wqkv_s = np.asarray(Wqkv, np.float32).copy()
    Wqkv_s[:, :DIM] *= DH ** -0.5  # fold q scaling into the projection
    Wqkv_bf = Wqkv_s.astype(bf)
    Wout_bf = np.asarray(Wout, np.float32).astype(bf)
    bout2 = np.asarray(bout, np.float32).reshape(1, DIM)

    ident_np = np.eye(128, dtype=bf)
    ident32_np = np.zeros((80, 65), dtype=np.float32)
    ident32_np[:65] = np.eye(65, dtype=np.float32)
    ind8_np = np.zeros((8, DIM), dtype=bf)
    for k in range(8):
        ind8_np[k, k * 64 : (k + 1) * 64] = 1.0

    # host-precomputed cls constants (mirrors the device bf16 dataflow)
    nc = _get_nc()
    in_maps = []
    for i in range(N_CORES):
        xcls_bf = x_pad[i, 0, :].astype(np.float32)  # bf16-quantized cls row
        qkv_cls = xcls_bf @ Wqkv_bf.astype(np.float32)  # [1536] fp32
        qkT_np = np.zeros((128, 8), dtype=bf)
        for m in range(8):
            qkT_np[:, m] = qkv_cls[m * 128 : (m + 1) * 128].astype(bf)
        v_cls_bf = qkv_cls[2 * DIM : 3 * DIM].astype(bf).astype(np.float32)
        vext_np = np.zeros((1, 8 * 65), dtype=bf)
        for hh in range(8):
            vext_np[0, hh * 65 : hh * 65 + 64] = v_cls_bf[hh * 64 : (hh + 1) * 64]
            vext_np[0, hh * 65 + 64] = 1.0
        # self-score from the bf16-quantized q/k (matches frame matmuls)
        qb = qkT_np[:, 0:4].astype(np.float32).T.reshape(DIM)
        kb = qkT_np[:, 4:8].astype(np.float32).T.reshape(DIM)
        s_self = (qb.reshape(8, 64) * kb.reshape(8, 64)).sum(axis=1)  # [8]
        a15 = 15.0 * np.exp(s_self)
        corr_np = np.zeros((8, 65), dtype=np.float32)
        corr_np[:, 0:64] = a15[:, None] * v_cls_bf.reshape(8, 64)
        corr_np[:, 64] = a15
        in_maps.append(
            {
                "x": x_pad[i],
                "wqkv": Wqkv_bf,
                "wout": Wout_bf,
                "bout": bout2,
                "ident": ident_np,
                "ident32": ident32_np,
                "ind8": ind8_np,
                "qkTcls": qkT_np,
                "vextcls": vext_np,
                "corr8": corr_np,
            }
        )
    res = run_bass_kernel_spmd(
        nc,
        in_maps,
        list(range(N_CORES)),
        trace=_trace,
        **(_trace_kwargs or {}),
    )
    out = np.stack([res.results[i]["out"] for i in range(N_CORES)], axis=0)
    if _trace:
        kernel.last_results = res
    return out
